# revision 1
# baseline (speedup 1.0000x reference)
"""Multi-head self-attention (B=2, T=2048, d_model=1024, 16 heads, causal)
on 8 trn2 NeuronCores.

Sharding: core c -> batch b=c//4, head-group g=c%4 (4 heads, d_model slice
of 256). Each core computes its heads' attention and a partial wo
projection [2048, 1024]; host sums the 4 partials per batch and adds bo.

Per-core pipeline (all matmul inputs bf16, fp32 PSUM accumulation):
  QT[dq,t] = (wq_s @ x^T)*0.125 + bq*0.125   (scale folded into weights)
  KT[dk,t] = wk_s @ x^T + bk
  V[t,dv]  = x @ wv_s^T + bv
  per head pair, per q-block j (512 wide):
    S^T[k,q] = K_h @ Q_h^T   (K=64 contraction, heads packed at partition
                              bases 0/64 -> concurrent row-group matmuls)
    E = exp(S^T)             (ACT, groups of 4 PSUM banks = 2 kt x 2 heads)
    diag tiles: E *= upper-tri mask
    O^T_aug = V_aug^T @ E    (V_aug = [64 ones cols | V_h cols] so PSUM rows
                              0-63 = replicated rowsums, rows 64-127 = O^T)
    OT_all = O^T * recip(rowsum)  (DVE reciprocal + tensor_tensor mul)
  P = OT_all^T @ wo_s^T      (partial output, fp32 out)
"""
import sys
sys.path.insert(0, "/opt/trn_rl_repo")
import numpy as np
import ml_dtypes

import concourse.bass as bass
import concourse.bacc as bacc
import concourse.tile as tile
import concourse.mybir as mybir
from concourse import bass_utils

BF16 = mybir.dt.bfloat16
F32 = mybir.dt.float32
EXP = mybir.ActivationFunctionType.Exp

T = 2048          # sequence length
DM = 1024         # d_model
DS = 256          # per-core d_model slice (4 heads x 64)
HD = 64           # head dim
NH = 4            # heads per core
KT128 = 16        # k tiles of 128 over T
QB = 512          # q block width
NJ = T // QB      # 4 q blocks
NCORES = 8

_CACHE = {}


def _build():
    nc = bacc.Bacc("TRN2", target_bir_lowering=False, debug=False,
                   enable_asserts=False, num_devices=NCORES)
    dram = {}
    for name, shape, dt in [
        ("xt", [DM, T], BF16),        # x[b]^T
        ("wqt", [DM, DS], BF16),      # wq.T[:, slice] * 0.125
        ("wkt", [DM, DS], BF16),
        ("wvt", [DM, DS], BF16),
        ("wot", [DS, DM], BF16),      # wo[:, slice].T
        ("bqc", [128, 2], F32),       # bq*0.125 as [128, m] columns
        ("bkc", [128, 2], F32),
        ("bv", [1, DS], BF16),
        ("tri", [128, 128], BF16),    # upper-tri (incl diag) ones
    ]:
        dram[name] = nc.dram_tensor(name, shape, dt, kind="ExternalInput").ap()
    p_out = nc.dram_tensor("p_out", [T, DM], F32, kind="ExternalOutput").ap()

    with tile.TileContext(nc) as tc:
        with tc.tile_pool(name="persist", bufs=1) as pp, \
             tc.tile_pool(name="epool", bufs=2) as ep, \
             tc.tile_pool(name="outp", bufs=4) as op, \
             tc.tile_pool(name="bcp", bufs=2) as bp, \
             tc.tile_pool(name="misc_ps", bufs=2, space="PSUM") as mp, \
             tc.tile_pool(name="st_ps", bufs=2, space="PSUM") as sp, \
             tc.tile_pool(name="ot_ps", bufs=1, space="PSUM") as tp:

            # ---- persistent SBUF ----
            xt = pp.tile([128, 8, T], BF16, name="xt")        # [p, kt8, t]
            wqt = pp.tile([128, 8, DS], BF16, name="wqt")
            wkt = pp.tile([128, 8, DS], BF16, name="wkt")
            wvt = pp.tile([128, 8, DS], BF16, name="wvt")
            wot = pp.tile([128, 2, DM], BF16, name="wot")
            qt = pp.tile([128, 2, T], BF16, name="qt")        # [p, dq-tile, t]
            kt = pp.tile([128, 2, T], BF16, name="kt")
            vaug = pp.tile([128, KT128, 512], BF16, name="vaug")
            ot_all = pp.tile([128, 2, T], BF16, name="ot_all")
            ones_row = pp.tile([1, 512], BF16, name="ones_row")
            bqc = pp.tile([128, 2], F32, name="bqc")
            bkc = pp.tile([128, 2], F32, name="bkc")
            bv_r = pp.tile([1, DS], BF16, name="bv_r")
            tri = pp.tile([128, 128], BF16, name="tri")

            nc.gpsimd.memset(ones_row, 1.0)
            # V_aug head block h: cols [128h, 128h+64) ones, [128h+64, +128) V
            for h in range(NH):
                nc.gpsimd.memset(vaug[:, :, 128 * h:128 * h + HD], 1.0)

            xt_dram = dram["xt"].rearrange("(kt p) t -> p kt t", p=128)
            # PE warmup: dummy matmuls on scratch during the input DMA wait so
            # HAM is at full clock when real matmuls start (results unread)
            warm = mp.tile([128, 512], F32, name="warm", tag="mp")
            for _ in range(24):
                nc.tensor.matmul(warm, lhsT=ones_row[0:1, 0:128],
                                 rhs=ones_row[0:1, :], start=True, stop=True)
            # x rows are 4KB contiguous per partition: load whole rows per k
            # on the sync queue; weights (wq first) on the gpsimd queue.
            nc.gpsimd.dma_start(out=bqc, in_=dram["bqc"])
            nc.gpsimd.dma_start(out=bkc, in_=dram["bkc"])
            nc.gpsimd.dma_start(out=bv_r, in_=dram["bv"])
            nc.gpsimd.dma_start(out=tri, in_=dram["tri"])
            for k in range(8):
                nc.sync.dma_start(out=xt[:, k, :], in_=xt_dram[:, k, :])
            for w_sb, w_nm in [(wqt, "wqt"), (wkt, "wkt"), (wvt, "wvt")]:
                for k in range(8):
                    nc.gpsimd.dma_start(
                        out=w_sb[:, k, :],
                        in_=dram[w_nm].rearrange("(kt p) d -> p kt d", p=128)[:, k, :])
            nc.gpsimd.dma_start(
                out=wot, in_=dram["wot"].rearrange("(kt p) d -> p kt d", p=128))

            # ---- streamed: proj(ts), then attention(j=ts), then wo(ts).
            # Attention for q-block j only needs Q/K/V through t-slice j, so
            # exp (ACT) and attention matmuls overlap the next projections.
            def proj(ts):
                t0 = ts * 512
                # QT / KT: out [dq 128, t 512]; bias fused into DVE copy
                for w_sb, b_c, dst in ((wqt, bqc, qt), (wkt, bkc, kt)):
                    for m in range(2):
                        ps = mp.tile([128, 512], F32, name="proj_ps", tag="mp")
                        for k in range(8):
                            nc.tensor.matmul(
                                ps, lhsT=w_sb[:, k, m * 128:(m + 1) * 128],
                                rhs=xt[:, k, t0:t0 + 512],
                                start=(k == 0), stop=(k == 7))
                        nc.vector.tensor_scalar_add(
                            dst[:, m, t0:t0 + 512], ps, b_c[:, m:m + 1])
                # V: out [t 128, dv 256] per 128-subtile
                for tt in range(4):
                    g = 4 * ts + tt
                    ps = mp.tile([128, 256], F32, name="v_ps", tag="mp")
                    for k in range(8):
                        nc.tensor.matmul(
                            ps, lhsT=xt[:, k, g * 128:(g + 1) * 128],
                            rhs=wvt[:, k, :], start=(k == 0), stop=False)
                    nc.tensor.matmul(
                        ps, lhsT=ones_row[0:1, 0:128], rhs=bv_r[0:1, :],
                        start=False, stop=True)
                    # scatter into vaug: head h -> cols [128h+64, 128h+128)
                    nc.vector.tensor_copy(
                        vaug[:, g, :].rearrange("p (h c) -> p h c", h=NH)[:, :, HD:],
                        ps.rearrange("p (h c) -> p h c", h=NH))

            def attention(j):
                q0 = j * QB
                nk = 4 * (j + 1)           # k-tiles of 128
                for H in range(2):          # head pair (2H, 2H+1)
                    # E[p, kt, hp, q]
                    e_t = ep.tile([128, KT128, 2, QB], BF16, name="e", tag="e")
                    ot = [tp.tile([128, QB], F32, name=f"ot{hp}", tag=f"ot{hp}")
                          for hp in range(2)]
                    for ktile in range(nk):
                        s = ktile - 4 * j       # >=0 on diag block
                        c0 = 128 * s if s >= 0 else 0
                        st = sp.tile([128, 2, 512], F32, name="st", tag="st")
                        for hp in range(2):
                            h = 2 * H + hp
                            r0 = (HD * h) % 128
                            mi = (HD * h) // 128
                            nc.tensor.matmul(
                                st[:, hp, c0:512],
                                lhsT=kt[r0:r0 + HD, mi,
                                        ktile * 128:(ktile + 1) * 128],
                                rhs=qt[r0:r0 + HD, mi, q0 + c0:q0 + QB],
                                start=True, stop=True)
                        nc.scalar.activation(
                            out=e_t[:, ktile, :, :], in_=st,
                            func=EXP, scale=1.0)
                        if s >= 0:
                            # mask both heads' diag tile in one strided op
                            dg = bass.AP(
                                tensor=e_t.tensor,
                                offset=e_t[:, ktile, 0,
                                           128 * s:128 * s + 1].offset,
                                ap=[e_t.ap[0], [QB, 2], [1, 128]])
                            trb = bass.AP(
                                tensor=tri.tensor, offset=tri.offset,
                                ap=[tri.ap[0], [0, 2], [1, 128]])
                            nc.vector.tensor_mul(dg, dg, trb)
                        for hp in range(2):
                            h = 2 * H + hp
                            nc.tensor.matmul(
                                ot[hp][:, c0:QB],
                                lhsT=vaug[:, ktile, 128 * h:128 * (h + 1)],
                                rhs=e_t[:, ktile, hp, c0:QB],
                                start=(ktile == 0), stop=(ktile == nk - 1))
                    for hp in range(2):
                        h = 2 * H + hp
                        rec = bp.tile([64, QB], F32, name="rec", tag="rec")
                        nc.vector.reciprocal_approx_fast(rec, ot[hp][0:64, :])
                        r0 = (HD * h) % 128
                        mi = (HD * h) // 128
                        nc.vector.tensor_mul(
                            ot_all[r0:r0 + HD, mi, q0:q0 + QB],
                            ot[hp][64:128, :], rec)

            def wo_block(j):
                q0 = j * QB
                for qq in range(4):
                    row = q0 + qq * 128
                    for n in range(2):
                        ps = tp.tile([128, 512], F32, name="wo_ps",
                                     tag=f"ot{(2 * qq + n) % 2}")
                        for kk in range(2):
                            nc.tensor.matmul(
                                ps, lhsT=ot_all[:, kk, row:row + 128],
                                rhs=wot[:, kk, n * 512:(n + 1) * 512],
                                start=(kk == 0), stop=(kk == 1))
                        ob = op.tile([128, 512], F32, name="ob", tag="ob")
                        nc.vector.tensor_copy(ob, ps)
                        dma_eng = nc.sync if n == 0 else nc.gpsimd
                        dma_eng.dma_start(
                            out=p_out[row:row + 128, n * 512:(n + 1) * 512],
                            in_=ob)

            for ts in range(4):
                proj(ts)
                attention(ts)
                wo_block(ts)
    nc.compile()
    return nc


def _prep_inputs(x, wq, bq, wk, bk, wv, bv, wo, bo):
    bf = ml_dtypes.bfloat16
    scale = np.float32(1.0 / np.sqrt(HD))
    tri = np.triu(np.ones((128, 128), np.float32)).astype(bf)
    in_maps = []
    for c in range(NCORES):
        b, g = c // 4, c % 4
        sl = slice(DS * g, DS * (g + 1))
        in_maps.append({
            "xt": np.ascontiguousarray(x[b].T).astype(bf),
            "wqt": np.ascontiguousarray(wq.T[:, sl] * scale).astype(bf),
            "wkt": np.ascontiguousarray(wk.T[:, sl]).astype(bf),
            "wvt": np.ascontiguousarray(wv.T[:, sl]).astype(bf),
            "wot": np.ascontiguousarray(wo[:, sl].T).astype(bf),
            "bqc": np.ascontiguousarray(
                (bq[sl] * scale).reshape(2, 128).T).astype(np.float32),
            "bkc": np.ascontiguousarray(
                bk[sl].reshape(2, 128).T).astype(np.float32),
            "bv": bv[sl].astype(bf).reshape(1, DS),
            "tri": tri,
        })
    return in_maps


TRACE = False
TRACE_DIR = None
LAST_RESULT = None


def kernel(x, wq, bq, wk, bk, wv, bv, wo, bo):
    global LAST_RESULT
    x, wq, bq, wk, bk, wv, bv, wo, bo = [
        np.asarray(a, np.float32)
        for a in (x, wq, bq, wk, bk, wv, bv, wo, bo)]
    if "nc" not in _CACHE:
        _CACHE["nc"] = _build()
    nc = _CACHE["nc"]
    in_maps = _prep_inputs(x, wq, bq, wk, bk, wv, bv, wo, bo)
    res = bass_utils.run_bass_kernel_spmd(
        nc, in_maps, core_ids=list(range(NCORES)), trace=TRACE,
        tmpdir=TRACE_DIR)
    LAST_RESULT = res
    out = np.empty((2, T, DM), np.float32)
    for b in range(2):
        acc = res.results[4 * b]["p_out"].astype(np.float32).copy()
        for g in range(1, 4):
            acc += res.results[4 * b + g]["p_out"]
        out[b] = acc + bo
    return out



# revision 2
# speedup vs baseline: 1.0607x; 1.0607x over previous
"""Multi-head self-attention (B=2, T=2048, d_model=1024, 16 heads, causal)
on 8 trn2 NeuronCores.

Sharding: core c -> batch b=c//4, head-group g=c%4 (4 heads, d_model slice
of 256). Each core computes its heads' attention and a partial wo
projection [2048, 1024] (bf16); host sums the 4 partials per batch and
adds bo + bv @ wo^T (the V-bias commutes through softmax since rows sum
to 1, so it is a host-side constant).

Per-core pipeline (all matmul inputs bf16, fp32 PSUM accumulation):
  QT[dq,t] = (wq_s @ x^T)*0.125 + bq*0.125   (scale folded into weights)
  KT[dk,t] = wk_s @ x^T + bk
  V[t,dv]  = x @ wv_s^T
  per head pair, per q-block j (512 wide):
    S^T[k,q] = K_h @ Q_h^T   (K=64 contraction, heads packed at partition
                              bases 0/64 -> concurrent row-group matmuls)
    E = exp(S^T)             (ACT, one call per (head-pair, ktile))
    diag tiles: E *= upper-tri mask
    O^T_aug = V_aug^T @ E    (V_aug = [64 ones cols | V_h cols] so PSUM rows
                              0-63 = replicated rowsums, rows 64-127 = O^T)
    OT_all = O^T * recip(rowsum)  (DVE reciprocal + tensor_tensor mul)
  P = OT_all^T @ wo_s^T      (partial output, bf16 out)

Scheduling: the per-ktile chain S(PE) -> exp(ACT) -> PV(PE) leaves PE
under-filled during attention (ACT needs ~1.15us per ktile vs ~0.64us of
attention PE work).  Projection and wo matmuls for neighboring q-blocks
are emitted as generator-based filler chunks interleaved between
attention ktile steps, keeping the PE queue busy while ACT grinds exp.
"""
import sys
sys.path.insert(0, "/opt/trn_rl_repo")
from collections import deque

import numpy as np
import ml_dtypes

import concourse.bass as bass
import concourse.bacc as bacc
import concourse.tile as tile
import concourse.mybir as mybir
from concourse import bass_utils

BF16 = mybir.dt.bfloat16
F32 = mybir.dt.float32
EXP = mybir.ActivationFunctionType.Exp

T = 2048          # sequence length
DM = 1024         # d_model
DS = 256          # per-core d_model slice (4 heads x 64)
HD = 64           # head dim
NH = 4            # heads per core
KT128 = 16        # k tiles of 128 over T
QB = 512          # q block width
NJ = T // QB      # 4 q blocks
NCORES = 8

_CACHE = {}


def _build():
    nc = bacc.Bacc("TRN2", target_bir_lowering=False, debug=False,
                   enable_asserts=False, num_devices=NCORES)
    dram = {}
    for name, shape, dt in [
        ("xt", [DM, T], BF16),        # x[b]^T
        ("wqt", [DM, DS], BF16),      # wq.T[:, slice] * 0.125
        ("wkt", [DM, DS], BF16),
        ("wvt", [DM, DS], BF16),
        ("wot", [DS, DM], BF16),      # wo[:, slice].T
        ("bqc", [128, 2], F32),       # bq*0.125 as [128, m] columns
        ("bkc", [128, 2], F32),
        ("tri", [128, 128], BF16),    # upper-tri (incl diag) ones
    ]:
        dram[name] = nc.dram_tensor(name, shape, dt, kind="ExternalInput").ap()
    p_out = nc.dram_tensor("p_out", [T, DM], BF16, kind="ExternalOutput").ap()

    with tile.TileContext(nc) as tc:
        with tc.tile_pool(name="persist", bufs=1) as pp, \
             tc.tile_pool(name="epool", bufs=3) as ep, \
             tc.tile_pool(name="outp", bufs=2) as op, \
             tc.tile_pool(name="bcp", bufs=2) as bp, \
             tc.tile_pool(name="misc_ps", bufs=2, space="PSUM") as mp, \
             tc.tile_pool(name="st_ps", bufs=2, space="PSUM") as sp, \
             tc.tile_pool(name="ot_ps", bufs=1, space="PSUM") as tp:

            # ---- persistent SBUF ----
            xt = pp.tile([128, 8, T], BF16, name="xt")        # [p, kt8, t]
            wqt = pp.tile([128, 8, DS], BF16, name="wqt")
            wkt = pp.tile([128, 8, DS], BF16, name="wkt")
            wvt = pp.tile([128, 8, DS], BF16, name="wvt")
            wot = pp.tile([128, 2, DM], BF16, name="wot")
            qt = pp.tile([128, 2, T], BF16, name="qt")        # [p, dq-tile, t]
            kt = pp.tile([128, 2, T], BF16, name="kt")
            vaug = pp.tile([128, KT128, 512], BF16, name="vaug")
            ot_all = pp.tile([128, 2, T], BF16, name="ot_all")
            ones_row = pp.tile([1, 512], BF16, name="ones_row")
            bqc = pp.tile([128, 2], F32, name="bqc")
            bkc = pp.tile([128, 2], F32, name="bkc")
            tri = pp.tile([128, 128], BF16, name="tri")

            # ones_row feeds the PE warmup; DVE is idle at kernel start.
            nc.vector.memset(ones_row, 1.0)
            # PE warmup: dummy matmuls on scratch during the input DMA wait so
            # HAM is at full clock when real matmuls start (results unread)
            warm = mp.tile([128, 512], F32, name="warm", tag="mp")
            for _ in range(10):
                nc.tensor.matmul(warm, lhsT=ones_row[0:1, 0:128],
                                 rhs=ones_row[0:1, :], start=True, stop=True)

            # Input DMA, chunked so proj(0) can start early.
            # sync queue: x t-slice 0, then wv (V(0) needs it), then x 1-3.
            # gpsimd queue: wq, wk first (Q/K(0) gate on them), small tensors,
            # wot last (first needed by wo(0) fillers in round 2).
            xt_dram = dram["xt"].rearrange("(kt p) t -> p kt t", p=128)
            nc.sync.dma_start(out=xt[:, :, 0:512], in_=xt_dram[:, :, 0:512])
            nc.sync.dma_start(
                out=wvt, in_=dram["wvt"].rearrange("(kt p) d -> p kt d", p=128))
            for ts in range(1, 4):
                t0 = ts * 512
                nc.sync.dma_start(out=xt[:, :, t0:t0 + 512],
                                  in_=xt_dram[:, :, t0:t0 + 512])
            nc.gpsimd.dma_start(
                out=wqt, in_=dram["wqt"].rearrange("(kt p) d -> p kt d", p=128))
            nc.gpsimd.dma_start(
                out=wkt, in_=dram["wkt"].rearrange("(kt p) d -> p kt d", p=128))
            nc.gpsimd.dma_start(out=bqc, in_=dram["bqc"])
            nc.gpsimd.dma_start(out=bkc, in_=dram["bkc"])
            nc.gpsimd.dma_start(out=tri, in_=dram["tri"])
            nc.gpsimd.dma_start(
                out=wot, in_=dram["wot"].rearrange("(kt p) d -> p kt d", p=128))
            # V_aug head block h: cols [128h, 128h+64) ones, [128h+64, +128) V.
            # After the DMA triggers so they don't delay the weight loads.
            for h in range(NH):
                nc.gpsimd.memset(vaug[:, :, 128 * h:128 * h + HD], 1.0)

            # ---- filler generators: projection / wo work emitted in small
            # chunks between attention ktile steps ----
            def gen_proj_qk(ts):
                t0 = ts * 512
                # QT / KT: out [dq 128, t 512]; bias fused into DVE copy
                for w_sb, b_c, dst in ((wqt, bqc, qt), (wkt, bkc, kt)):
                    for m in range(2):
                        ps = mp.tile([128, 512], F32, name="proj_ps", tag="mp")
                        for k in range(8):
                            nc.tensor.matmul(
                                ps, lhsT=w_sb[:, k, m * 128:(m + 1) * 128],
                                rhs=xt[:, k, t0:t0 + 512],
                                start=(k == 0), stop=(k == 7))
                            if k % 2 == 1 and k < 7:
                                yield
                        nc.vector.tensor_scalar_add(
                            dst[:, m, t0:t0 + 512], ps, b_c[:, m:m + 1])
                        yield

            def gen_proj_v(ts):
                # V: out [t 128, dv 256] per 128-subtile
                for tt in range(4):
                    g = 4 * ts + tt
                    ps = mp.tile([128, 256], F32, name="v_ps", tag="mp")
                    for k in range(8):
                        nc.tensor.matmul(
                            ps, lhsT=xt[:, k, g * 128:(g + 1) * 128],
                            rhs=wvt[:, k, :], start=(k == 0), stop=(k == 7))
                        if k % 4 == 3 and k < 7:
                            yield
                    # scatter into vaug: head h -> cols [128h+64, 128h+128)
                    nc.vector.tensor_copy(
                        vaug[:, g, :].rearrange("p (h c) -> p h c", h=NH)[:, :, HD:],
                        ps.rearrange("p (h c) -> p h c", h=NH))
                    yield

            def gen_wo(j):
                q0 = j * QB
                ob = op.tile([128, 4, DM], BF16, name="ob", tag="ob")
                for qq in range(4):
                    row = q0 + qq * 128
                    for n in range(2):
                        ps = mp.tile([128, 512], F32, name="wo_ps", tag="mp")
                        for kk in range(2):
                            nc.tensor.matmul(
                                ps, lhsT=ot_all[:, kk, row:row + 128],
                                rhs=wot[:, kk, n * 512:(n + 1) * 512],
                                start=(kk == 0), stop=(kk == 1))
                        nc.vector.tensor_copy(ob[:, qq, n * 512:(n + 1) * 512],
                                              ps)
                        yield
                dma_eng = nc.sync if j % 2 == 0 else nc.gpsimd
                dma_eng.dma_start(
                    out=p_out[q0:q0 + 512, :].rearrange(
                        "(q p) d -> p q d", p=128),
                    in_=ob)
                yield

            fillers = deque()

            def fill(n):
                while n > 0 and fillers:
                    try:
                        next(fillers[0])
                        n -= 1
                    except StopIteration:
                        fillers.popleft()

            def drain():
                while fillers:
                    fill(64)

            def attention(j):
                q0 = j * QB
                nk = 4 * (j + 1)           # k-tiles of 128
                for H in range(2):          # head pair (2H, 2H+1)
                    ot = [tp.tile([128, QB], F32, name=f"ot{hp}", tag=f"ot{hp}")
                          for hp in range(2)]
                    for ktile in range(nk):
                        s = ktile - 4 * j       # >=0 on diag block
                        c0 = 128 * s if s >= 0 else 0
                        st = sp.tile([128, 2, 512], F32, name="st", tag="st")
                        e_t = ep.tile([128, 2, QB], BF16, name="e", tag="e")
                        for hp in range(2):
                            h = 2 * H + hp
                            r0 = (HD * h) % 128
                            mi = (HD * h) // 128
                            nc.tensor.matmul(
                                st[:, hp, c0:512],
                                lhsT=kt[r0:r0 + HD, mi,
                                        ktile * 128:(ktile + 1) * 128],
                                rhs=qt[r0:r0 + HD, mi, q0 + c0:q0 + QB],
                                start=True, stop=True)
                        nc.scalar.activation(
                            out=e_t, in_=st, func=EXP, scale=1.0)
                        if s >= 0:
                            # mask both heads' diag tile in one strided op
                            dg = e_t[:, :, c0:c0 + 128]
                            trb = bass.AP(
                                tensor=tri.tensor, offset=tri.offset,
                                ap=[tri.ap[0], [0, 2], [1, 128]])
                            nc.vector.tensor_mul(dg, dg, trb)
                        fill(2)
                        for hp in range(2):
                            h = 2 * H + hp
                            nc.tensor.matmul(
                                ot[hp][:, c0:QB],
                                lhsT=vaug[:, ktile, 128 * h:128 * (h + 1)],
                                rhs=e_t[:, hp, c0:QB],
                                start=(ktile == 0), stop=(ktile == nk - 1))
                    for hp in range(2):
                        h = 2 * H + hp
                        rec = bp.tile([64, QB], F32, name="rec", tag="rec")
                        nc.vector.reciprocal_approx_fast(rec, ot[hp][0:64, :])
                        r0 = (HD * h) % 128
                        mi = (HD * h) // 128
                        nc.vector.tensor_mul(
                            ot_all[r0:r0 + HD, mi, q0:q0 + QB],
                            ot[hp][64:128, :], rec)

            # ---- schedule ----
            # Round 0: proj(0) emitted directly (nothing to overlap with yet).
            for g in (gen_proj_qk(0), gen_proj_v(0)):
                for _ in g:
                    pass
            # Rounds j=0..3: attention(j) with proj(j+1) / wo(j-1) as filler.
            for j in range(NJ):
                if j + 1 < NJ:
                    fillers.append(gen_proj_qk(j + 1))
                    fillers.append(gen_proj_v(j + 1))
                if j - 1 >= 0:
                    fillers.append(gen_wo(j - 1))
                attention(j)
            drain()
            for _ in gen_wo(NJ - 1):
                pass
    nc.compile()
    return nc


def _prep_inputs(x, wq, bq, wk, bk, wv, wo):
    bf = ml_dtypes.bfloat16
    scale = np.float32(1.0 / np.sqrt(HD))
    tri = np.triu(np.ones((128, 128), np.float32)).astype(bf)
    in_maps = []
    for c in range(NCORES):
        b, g = c // 4, c % 4
        sl = slice(DS * g, DS * (g + 1))
        in_maps.append({
            "xt": np.ascontiguousarray(x[b].T).astype(bf),
            "wqt": np.ascontiguousarray(wq.T[:, sl] * scale).astype(bf),
            "wkt": np.ascontiguousarray(wk.T[:, sl]).astype(bf),
            "wvt": np.ascontiguousarray(wv.T[:, sl]).astype(bf),
            "wot": np.ascontiguousarray(wo[:, sl].T).astype(bf),
            "bqc": np.ascontiguousarray(
                (bq[sl] * scale).reshape(2, 128).T).astype(np.float32),
            "bkc": np.ascontiguousarray(
                bk[sl].reshape(2, 128).T).astype(np.float32),
            "tri": tri,
        })
    return in_maps


TRACE = False
TRACE_DIR = None
LAST_RESULT = None


def kernel(x, wq, bq, wk, bk, wv, bv, wo, bo):
    global LAST_RESULT
    x, wq, bq, wk, bk, wv, bv, wo, bo = [
        np.asarray(a, np.float32)
        for a in (x, wq, bq, wk, bk, wv, bv, wo, bo)]
    if "nc" not in _CACHE:
        _CACHE["nc"] = _build()
    nc = _CACHE["nc"]
    in_maps = _prep_inputs(x, wq, bq, wk, bk, wv, wo)
    res = bass_utils.run_bass_kernel_spmd(
        nc, in_maps, core_ids=list(range(NCORES)), trace=TRACE,
        tmpdir=TRACE_DIR)
    LAST_RESULT = res
    # bv commutes through softmax (rows sum to 1): out += bv @ wo^T + bo.
    const_row = (bv.astype(np.float64) @ wo.T.astype(np.float64) +
                 bo.astype(np.float64)).astype(np.float32)
    out = np.empty((2, T, DM), np.float32)
    for b in range(2):
        acc = res.results[4 * b]["p_out"].astype(np.float32)
        for g in range(1, 4):
            acc = acc + res.results[4 * b + g]["p_out"].astype(np.float32)
        out[b] = acc + const_row
    return out


# revision 5
# speedup vs baseline: 1.1119x; 1.0482x over previous
"""Multi-head self-attention (B=2, T=2048, d_model=1024, 16 heads, causal)
on 8 trn2 NeuronCores.

Sharding: core c -> batch b=c//4, head-group g=c%4 (4 heads, d_model slice
of 256). Each core computes its heads' attention and a partial wo
projection [2048, 1024] (bf16); host sums the 4 partials per batch and
adds bo + bv @ wo^T (the V-bias commutes through softmax since rows sum
to 1, so it is a host-side constant).

Per-core pipeline (all matmul inputs bf16, fp32 PSUM accumulation):
  QT[dq,t] = (wq_s @ x^T)*0.125 + bq*0.125   (scale folded into weights)
  KT[dk,t] = wk_s @ x^T + bk
  V[t,dv]  = x @ wv_s^T
  per head pair, per q-block j (512 wide):
    S^T[k,q] = K_h @ Q_h^T   (K=64 contraction, heads packed at partition
                              bases 0/64 -> concurrent row-group matmuls)
    E = exp(S^T)             (ACT, one call per (head-pair, ktile))
    diag tiles: E *= upper-tri mask
    O^T_aug = V_aug^T @ E    (V_aug = [64 ones cols | V_h cols] so PSUM rows
                              0-63 = replicated rowsums, rows 64-127 = O^T)
    OT_all = O^T * recip(rowsum)  (DVE reciprocal + tensor_tensor mul)
  P = OT_all^T @ wo_s^T      (partial output, bf16 out)

Scheduling: the per-ktile chain S(PE) -> exp(ACT) -> PV(PE) leaves PE
under-filled during attention (ACT needs ~1.15us per ktile vs ~0.64us of
attention PE work).  Projection and wo matmuls for neighboring q-blocks
are emitted as generator-based filler chunks (~0.4-0.9us of PE work per
chunk) interleaved between attention ktile steps, paced so each t-slice
projection lands just before the attention round that consumes it.
Round 0 starts attention immediately after the minimal prefix
(Q m0, K m0, V tile 0), with the rest of proj(0) as its filler.
"""
import sys
sys.path.insert(0, "/opt/trn_rl_repo")
from collections import deque

import numpy as np
import ml_dtypes

import concourse.bass as bass
import concourse.bacc as bacc
import concourse.tile as tile
import concourse.mybir as mybir
from concourse import bass_utils

BF16 = mybir.dt.bfloat16
F32 = mybir.dt.float32
EXP = mybir.ActivationFunctionType.Exp

T = 2048          # sequence length
DM = 1024         # d_model
DS = 256          # per-core d_model slice (4 heads x 64)
HD = 64           # head dim
NH = 4            # heads per core
KT128 = 16        # k tiles of 128 over T
QB = 512          # q block width
NJ = T // QB      # 4 q blocks
NCORES = 8

_CACHE = {}


def _build():
    nc = bacc.Bacc("TRN2", target_bir_lowering=False, debug=False,
                   enable_asserts=False, num_devices=NCORES)
    dram = {}
    for name, shape, dt in [
        ("xt", [DM, T], BF16),        # x[b]^T
        ("wqt", [DM, DS], BF16),      # wq.T[:, slice] * 0.125
        ("wkt", [DM, DS], BF16),
        ("wvt", [DM, DS], BF16),
        ("wot", [DS, DM], BF16),      # wo[:, slice].T
        ("bqc", [128, 2], F32),       # bq*0.125 as [128, m] columns
        ("bkc", [128, 2], F32),
        ("tri", [128, 128], BF16),    # upper-tri (incl diag) ones
    ]:
        dram[name] = nc.dram_tensor(name, shape, dt, kind="ExternalInput").ap()
    p_out = nc.dram_tensor("p_out", [T, DM], BF16, kind="ExternalOutput").ap()

    with tile.TileContext(nc) as tc:
        with tc.tile_pool(name="persist", bufs=1) as pp, \
             tc.tile_pool(name="epool", bufs=3) as ep, \
             tc.tile_pool(name="outp", bufs=2) as op, \
             tc.tile_pool(name="bcp", bufs=2) as bp, \
             tc.tile_pool(name="misc_ps", bufs=2, space="PSUM") as mp, \
             tc.tile_pool(name="st_ps", bufs=2, space="PSUM") as sp, \
             tc.tile_pool(name="ot_ps", bufs=1, space="PSUM") as tp:

            # ---- persistent SBUF ----
            xt = pp.tile([128, 8, T], BF16, name="xt")        # [p, kt8, t]
            wqt = pp.tile([128, 8, DS], BF16, name="wqt")
            wkt = pp.tile([128, 8, DS], BF16, name="wkt")
            wvt = pp.tile([128, 8, DS], BF16, name="wvt")
            wot = pp.tile([128, 2, DM], BF16, name="wot")
            qt = pp.tile([128, 2, T], BF16, name="qt")        # [p, dq-tile, t]
            kt = pp.tile([128, 2, T], BF16, name="kt")
            vaug = pp.tile([128, KT128, 512], BF16, name="vaug")
            ot_all = pp.tile([128, 2, T], BF16, name="ot_all")
            ones_row = pp.tile([1, 512], BF16, name="ones_row")
            bqc = pp.tile([128, 2], F32, name="bqc")
            bkc = pp.tile([128, 2], F32, name="bkc")
            tri = pp.tile([128, 128], BF16, name="tri")

            # ones_row feeds the PE warmup; DVE is otherwise idle at start.
            nc.vector.memset(ones_row, 1.0)
            # PE warmup: dummy matmuls on scratch during the input DMA wait so
            # HAM is at full clock when real matmuls start (results unread)
            warm = mp.tile([128, 512], F32, name="warm", tag="mp")
            for _ in range(7):
                nc.tensor.matmul(warm, lhsT=ones_row[0:1, 0:128],
                                 rhs=ones_row[0:1, :], start=True, stop=True)

            # Input DMA: first-wave triggers spread across four engines so the
            # tensors gating proj(0) (wq, wk, x t-slice 0) land ASAP.
            xt_dram = dram["xt"].rearrange("(kt p) t -> p kt t", p=128)

            def w_dram(nm):
                return dram[nm].rearrange("(kt p) d -> p kt d", p=128)

            nc.sync.dma_start(out=xt[:, :, 0:512], in_=xt_dram[:, :, 0:512])
            nc.sync.dma_start(out=wvt, in_=w_dram("wvt"))
            for ts in range(1, 4):
                t0 = ts * 512
                nc.sync.dma_start(out=xt[:, :, t0:t0 + 512],
                                  in_=xt_dram[:, :, t0:t0 + 512])
            nc.gpsimd.dma_start(out=wqt, in_=w_dram("wqt"))
            nc.gpsimd.dma_start(out=wot, in_=w_dram("wot"))
            nc.scalar.dma_start(out=wkt, in_=w_dram("wkt"))
            nc.scalar.dma_start(out=bqc, in_=dram["bqc"])
            nc.scalar.dma_start(out=bkc, in_=dram["bkc"])
            nc.scalar.dma_start(out=tri, in_=dram["tri"])
            # V_aug head block h: cols [128h, 128h+64) ones, [128h+64, +128) V.
            # After the DMA triggers so they don't delay the weight loads.
            for h in range(NH):
                nc.gpsimd.memset(vaug[:, :, 128 * h:128 * h + HD], 1.0)

            # ---- emission helpers ----
            def proj_qk_m(w_sb, b_c, dst, ts, m, chunk=99):
                """One [128,512] psum tile of the Q/K projection; yields every
                `chunk` contraction matmuls."""
                t0 = ts * 512
                ps = mp.tile([128, 512], F32, name="proj_ps", tag="mp")
                for k in range(8):
                    nc.tensor.matmul(
                        ps, lhsT=w_sb[:, k, m * 128:(m + 1) * 128],
                        rhs=xt[:, k, t0:t0 + 512],
                        start=(k == 0), stop=(k == 7))
                    if k % chunk == chunk - 1 and k < 7:
                        yield
                nc.vector.tensor_scalar_add(
                    dst[:, m, t0:t0 + 512], ps, b_c[:, m:m + 1])
                yield

            def proj_v_tt(g):
                """V projection for t-subtile g (128 rows): 8 matmuls + the
                scatter into vaug."""
                ps = mp.tile([128, 256], F32, name="v_ps", tag="mp")
                for k in range(8):
                    nc.tensor.matmul(
                        ps, lhsT=xt[:, k, g * 128:(g + 1) * 128],
                        rhs=wvt[:, k, :], start=(k == 0), stop=(k == 7))
                    if k == 3:
                        yield
                # scatter into vaug: head h -> cols [128h+64, 128h+128)
                nc.vector.tensor_copy(
                    vaug[:, g, :].rearrange("p (h c) -> p h c", h=NH)[:, :, HD:],
                    ps.rearrange("p (h c) -> p h c", h=NH))
                yield

            def gen_proj_qk(ts):
                for w_sb, b_c, dst in ((wqt, bqc, qt), (wkt, bkc, kt)):
                    for m in range(2):
                        yield from proj_qk_m(w_sb, b_c, dst, ts, m, chunk=2)

            def gen_proj_qk_pair(ts_a, ts_b):
                """Q/K projections for two t-slices with shared stationary
                weights: consecutive matmuls reuse the same lhsT."""
                for w_sb, b_c, dst in ((wqt, bqc, qt), (wkt, bkc, kt)):
                    for m in range(2):
                        ps_a = mp.tile([128, 512], F32, name="ppsa", tag="mp")
                        ps_b = mp.tile([128, 512], F32, name="ppsb", tag="mp")
                        for k in range(8):
                            w_ap = w_sb[:, k, m * 128:(m + 1) * 128]
                            for ps, ts in ((ps_a, ts_a), (ps_b, ts_b)):
                                t0 = ts * 512
                                nc.tensor.matmul(
                                    ps, lhsT=w_ap,
                                    rhs=xt[:, k, t0:t0 + 512],
                                    start=(k == 0), stop=(k == 7))
                            if k % 2 == 1 and k < 7:
                                yield
                        for ps, ts in ((ps_a, ts_a), (ps_b, ts_b)):
                            nc.vector.tensor_scalar_add(
                                dst[:, m, ts * 512:ts * 512 + 512],
                                ps, b_c[:, m:m + 1])
                        yield

            def gen_proj_v(ts):
                for tt in range(4):
                    yield from proj_v_tt(4 * ts + tt)

            def gen_wo(j):
                q0 = j * QB
                last = j == NJ - 1
                ob = op.tile([128, 4, DM], BF16, name="ob", tag="ob")
                for qq in range(4):
                    row = q0 + qq * 128
                    for n in range(2):
                        ps = mp.tile([128, 512], F32, name="wo_ps", tag="mp")
                        for kk in range(2):
                            nc.tensor.matmul(
                                ps, lhsT=ot_all[:, kk, row:row + 128],
                                rhs=wot[:, kk, n * 512:(n + 1) * 512],
                                start=(kk == 0), stop=(kk == 1))
                        nc.vector.tensor_copy(ob[:, qq, n * 512:(n + 1) * 512],
                                              ps)
                        yield
                    if last:
                        # tail round: stream the output per 128-row chunk so
                        # the final DMA only trails the last copy
                        dma_eng = nc.sync if qq % 2 == 0 else nc.gpsimd
                        dma_eng.dma_start(
                            out=p_out[row:row + 128, :], in_=ob[:, qq, :])
                if not last:
                    dma_eng = nc.sync if j % 2 == 0 else nc.gpsimd
                    dma_eng.dma_start(
                        out=p_out[q0:q0 + 512, :].rearrange(
                            "(q p) d -> p q d", p=128),
                        in_=ob)
                yield

            # Global filler queue: (key, generator), drained strictly FIFO.
            fillers = deque()

            def fill(n):
                while n > 0 and fillers:
                    try:
                        next(fillers[0][1])
                        n -= 1
                    except StopIteration:
                        fillers.popleft()

            def force(key):
                """Drain fillers until generator `key` has been exhausted."""
                while any(k == key for k, _ in fillers):
                    try:
                        next(fillers[0][1])
                    except StopIteration:
                        fillers.popleft()

            def attention(j, fill_fn):
                q0 = j * QB
                nk = 4 * (j + 1)           # k-tiles of 128
                for H in range(2):          # head pair (2H, 2H+1)
                    ot = [tp.tile([128, QB], F32, name=f"ot{hp}", tag=f"ot{hp}")
                          for hp in range(2)]
                    for ktile in range(nk):
                        s = ktile - 4 * j       # >=0 on diag block
                        c0 = 128 * s if s >= 0 else 0
                        st = sp.tile([128, 2, 512], F32, name="st", tag="st")
                        e_t = ep.tile([128, 2, QB], BF16, name="e", tag="e")
                        for hp in range(2):
                            h = 2 * H + hp
                            r0 = (HD * h) % 128
                            mi = (HD * h) // 128
                            nc.tensor.matmul(
                                st[:, hp, c0:512],
                                lhsT=kt[r0:r0 + HD, mi,
                                        ktile * 128:(ktile + 1) * 128],
                                rhs=qt[r0:r0 + HD, mi, q0 + c0:q0 + QB],
                                start=True, stop=True)
                        nc.scalar.activation(
                            out=e_t, in_=st, func=EXP, scale=1.0)
                        if s >= 0:
                            # mask both heads' diag tile in one strided op
                            dg = e_t[:, :, c0:c0 + 128]
                            trb = bass.AP(
                                tensor=tri.tensor, offset=tri.offset,
                                ap=[tri.ap[0], [0, 2], [1, 128]])
                            nc.vector.tensor_mul(dg, dg, trb)
                        fill_fn(j, H, ktile)
                        for hp in range(2):
                            h = 2 * H + hp
                            nc.tensor.matmul(
                                ot[hp][:, c0:QB],
                                lhsT=vaug[:, ktile, 128 * h:128 * (h + 1)],
                                rhs=e_t[:, hp, c0:QB],
                                start=(ktile == 0), stop=(ktile == nk - 1))
                    for hp in range(2):
                        h = 2 * H + hp
                        rec = bp.tile([64, QB], F32, name="rec", tag="rec")
                        nc.vector.reciprocal_approx_fast(rec, ot[hp][0:64, :])
                        r0 = (HD * h) % 128
                        mi = (HD * h) // 128
                        nc.vector.tensor_mul(
                            ot_all[r0:r0 + HD, mi, q0:q0 + QB],
                            ot[hp][64:128, :], rec)

            # ---- schedule ----
            # Round 0 prefix: Q m0, K m0, V g=0 -> attention(0) H0 can start;
            # the rest of proj(0) rides along as round-0 filler.
            for _ in proj_qk_m(wqt, bqc, qt, 0, 0):
                pass
            for _ in proj_qk_m(wkt, bkc, kt, 0, 0):
                pass
            for _ in proj_v_tt(0):
                pass
            local = deque()
            for g in (proj_v_tt(1), proj_v_tt(2), proj_v_tt(3),
                      proj_qk_m(wqt, bqc, qt, 0, 1, chunk=4),
                      proj_qk_m(wkt, bkc, kt, 0, 1, chunk=4)):
                local.append(("r0", g))

            def fill_r0(j, H, ktile):
                n = 3
                while n > 0 and local:
                    try:
                        next(local[0][1])
                        n -= 1
                    except StopIteration:
                        local.popleft()
                if not local:
                    fill(1)

            fillers.append((("qk", 1), gen_proj_qk(1)))
            fillers.append((("v", 1), gen_proj_v(1)))
            fillers.append((("qk", 3), gen_proj_qk_pair(2, 3)))
            fillers.append((("v", 2), gen_proj_v(2)))
            fillers.append((("v", 3), gen_proj_v(3)))

            def fill_main(j, H, ktile):
                if H == 0 and ktile == 4 * j:
                    force(("v", j))   # vaug t-slice j gate (usually a no-op)
                step = H * 4 * (j + 1) + ktile
                fill(2 if (j == 1 and step < 4) else 1)

            attention(0, fill_r0)
            while local:
                fill_r0(0, 0, 0)
            for j in range(1, NJ):
                force(("qk", j if j != 2 else 3))
                fillers.append((("wo", j - 1), gen_wo(j - 1)))
                attention(j, fill_main)
            while fillers:
                fill(64)
            for _ in gen_wo(NJ - 1):
                pass
    nc.compile()
    return nc


def _prep_inputs(x, wq, bq, wk, bk, wv, wo):
    bf = ml_dtypes.bfloat16
    scale = np.float32(1.0 / np.sqrt(HD))
    tri = np.triu(np.ones((128, 128), np.float32)).astype(bf)
    in_maps = []
    for c in range(NCORES):
        b, g = c // 4, c % 4
        sl = slice(DS * g, DS * (g + 1))
        in_maps.append({
            "xt": np.ascontiguousarray(x[b].T).astype(bf),
            "wqt": np.ascontiguousarray(wq.T[:, sl] * scale).astype(bf),
            "wkt": np.ascontiguousarray(wk.T[:, sl]).astype(bf),
            "wvt": np.ascontiguousarray(wv.T[:, sl]).astype(bf),
            "wot": np.ascontiguousarray(wo[:, sl].T).astype(bf),
            "bqc": np.ascontiguousarray(
                (bq[sl] * scale).reshape(2, 128).T).astype(np.float32),
            "bkc": np.ascontiguousarray(
                bk[sl].reshape(2, 128).T).astype(np.float32),
            "tri": tri,
        })
    return in_maps


TRACE = False
TRACE_DIR = None
LAST_RESULT = None


def kernel(x, wq, bq, wk, bk, wv, bv, wo, bo):
    global LAST_RESULT
    x, wq, bq, wk, bk, wv, bv, wo, bo = [
        np.asarray(a, np.float32)
        for a in (x, wq, bq, wk, bk, wv, bv, wo, bo)]
    if "nc" not in _CACHE:
        _CACHE["nc"] = _build()
    nc = _CACHE["nc"]
    in_maps = _prep_inputs(x, wq, bq, wk, bk, wv, wo)
    res = bass_utils.run_bass_kernel_spmd(
        nc, in_maps, core_ids=list(range(NCORES)), trace=TRACE,
        tmpdir=TRACE_DIR)
    LAST_RESULT = res
    # bv commutes through softmax (rows sum to 1): out += bv @ wo^T + bo.
    const_row = (bv.astype(np.float64) @ wo.T.astype(np.float64) +
                 bo.astype(np.float64)).astype(np.float32)
    out = np.empty((2, T, DM), np.float32)
    for b in range(2):
        acc = res.results[4 * b]["p_out"].astype(np.float32)
        for g in range(1, 4):
            acc = acc + res.results[4 * b + g]["p_out"].astype(np.float32)
        out[b] = acc + const_row
    return out


# revision 6
# speedup vs baseline: 1.1437x; 1.0286x over previous
"""Multi-head self-attention (B=2, T=2048, d_model=1024, 16 heads, causal)
on 8 trn2 NeuronCores.

Sharding: core c -> batch b=c//4, head-group g=c%4 (4 heads, d_model slice
of 256). Each core computes its heads' attention and a partial wo
projection [2048, 1024] (bf16); host sums the 4 partials per batch and
adds bo + bv @ wo^T (the V-bias commutes through softmax since rows sum
to 1, so it is a host-side constant).

Per-core pipeline (all matmul inputs bf16, fp32 PSUM accumulation):
  QT[dq,t] = (wq_s @ x^T)*0.125 + bq*0.125   (scale folded into weights)
  KT[dk,t] = wk_s @ x^T + bk
  V[t,dv]  = x @ wv_s^T
  per head pair, per q-block j (512 wide):
    S^T[k,q] = K_h @ Q_h^T   (K=64 contraction, heads packed at partition
                              bases 0/64 -> concurrent row-group matmuls)
    E = exp(S^T)             (ACT, one call per (head-pair, ktile))
    diag tiles: E *= upper-tri mask
    O^T_aug = V_aug^T @ E    (V_aug = [64 ones cols | V_h cols] so PSUM rows
                              0-63 = replicated rowsums, rows 64-127 = O^T)
    OT_all = O^T * recip(rowsum)  (DVE reciprocal + tensor_tensor mul)
  P = OT_all^T @ wo_s^T      (partial output, bf16 out)

Scheduling: the per-ktile chain S(PE) -> exp(ACT) -> PV(PE) leaves PE
under-filled during attention (ACT needs ~1.15us per ktile vs ~0.64us of
attention PE work).  Projection and wo matmuls for neighboring q-blocks
are emitted as generator-based filler chunks (~0.4us of PE work per
chunk) interleaved between attention ktile steps; wo work is held back
to round 3, the most ACT-bound stretch.  x is staged t-slice-major
(both DRAM and SBUF) so each input DMA moves 8KB-contiguous lines per
partition at full rate and projections of t-slice 0 start ~10us in.
"""
import sys
sys.path.insert(0, "/opt/trn_rl_repo")
from collections import deque

import numpy as np
import ml_dtypes

import concourse.bass as bass
import concourse.bacc as bacc
import concourse.tile as tile
import concourse.mybir as mybir
from concourse import bass_utils

BF16 = mybir.dt.bfloat16
F32 = mybir.dt.float32
EXP = mybir.ActivationFunctionType.Exp

T = 2048          # sequence length
DM = 1024         # d_model
DS = 256          # per-core d_model slice (4 heads x 64)
HD = 64           # head dim
NH = 4            # heads per core
KT128 = 16        # k tiles of 128 over T
QB = 512          # q block width
NJ = T // QB      # 4 q blocks
NCORES = 8

_CACHE = {}


def _build():
    nc = bacc.Bacc("TRN2", target_bir_lowering=False, debug=False,
                   enable_asserts=False, num_devices=NCORES)
    dram = {}
    for name, shape, dt in [
        ("xt", [NJ, 128, 8, 512], BF16),  # x[b]^T, t-slice-major chunks
        ("wqt", [DM, DS], BF16),      # wq.T[:, slice] * 0.125
        ("wkt", [DM, DS], BF16),
        ("wvt", [DM, DS], BF16),
        ("wot", [DS, DM], BF16),      # wo[:, slice].T
        ("bqc", [128, 2], F32),       # bq*0.125 as [128, m] columns
        ("bkc", [128, 2], F32),
        ("tri", [128, 128], BF16),    # upper-tri (incl diag) ones
    ]:
        dram[name] = nc.dram_tensor(name, shape, dt, kind="ExternalInput").ap()
    p_out = nc.dram_tensor("p_out", [T, DM], BF16, kind="ExternalOutput").ap()

    with tile.TileContext(nc) as tc:
        with tc.tile_pool(name="persist", bufs=1) as pp, \
             tc.tile_pool(name="epool", bufs=3) as ep, \
             tc.tile_pool(name="outp", bufs=2) as op, \
             tc.tile_pool(name="bcp", bufs=2) as bp, \
             tc.tile_pool(name="misc_ps", bufs=2, space="PSUM") as mp, \
             tc.tile_pool(name="st_ps", bufs=2, space="PSUM") as sp, \
             tc.tile_pool(name="ot_ps", bufs=1, space="PSUM") as tp:

            # ---- persistent SBUF ----
            xt = pp.tile([128, NJ, 8, 512], BF16, name="xt")  # [p, ts, kt8, c]
            wqt = pp.tile([128, 8, DS], BF16, name="wqt")
            wkt = pp.tile([128, 8, DS], BF16, name="wkt")
            wvt = pp.tile([128, 8, DS], BF16, name="wvt")
            wot = pp.tile([128, 2, DM], BF16, name="wot")
            qt = pp.tile([128, 2, T], BF16, name="qt")        # [p, dq-tile, t]
            kt = pp.tile([128, 2, T], BF16, name="kt")
            vaug = pp.tile([128, KT128, 512], BF16, name="vaug")
            ot_all = pp.tile([128, 2, T], BF16, name="ot_all")
            ones_row = pp.tile([1, 512], BF16, name="ones_row")
            bqc = pp.tile([128, 2], F32, name="bqc")
            bkc = pp.tile([128, 2], F32, name="bkc")
            tri = pp.tile([128, 128], BF16, name="tri")

            # ones_row feeds the PE warmup; DVE is otherwise idle at start.
            nc.vector.memset(ones_row, 1.0)
            # PE warmup: dummy matmuls on scratch during the input DMA wait so
            # HAM is at full clock when real matmuls start (results unread)
            warm = mp.tile([128, 512], F32, name="warm", tag="mp")
            for _ in range(7):
                nc.tensor.matmul(warm, lhsT=ones_row[0:1, 0:128],
                                 rhs=ones_row[0:1, :], start=True, stop=True)

            # Input DMA: first-wave triggers spread across three engines so the
            # tensors gating proj(0) (wq, wk, x t-slice 0) land ASAP.
            def w_dram(nm):
                return dram[nm].rearrange("(kt p) d -> p kt d", p=128)

            nc.sync.dma_start(out=xt[:, 0], in_=dram["xt"][0])
            nc.sync.dma_start(out=wvt, in_=w_dram("wvt"))
            nc.sync.dma_start(
                out=xt[:, 1:4],
                in_=dram["xt"][1:4].rearrange("ts p kt c -> p ts kt c"))
            nc.gpsimd.dma_start(out=wqt, in_=w_dram("wqt"))
            nc.gpsimd.dma_start(out=wot, in_=w_dram("wot"))
            nc.scalar.dma_start(out=wkt, in_=w_dram("wkt"))
            nc.scalar.dma_start(out=bqc, in_=dram["bqc"])
            nc.scalar.dma_start(out=bkc, in_=dram["bkc"])
            nc.scalar.dma_start(out=tri, in_=dram["tri"])
            # V_aug head block h: cols [128h, 128h+64) ones, [128h+64, +128) V.
            # After the DMA triggers so they don't delay the weight loads.
            for h in range(NH):
                nc.gpsimd.memset(vaug[:, :, 128 * h:128 * h + HD], 1.0)

            # ---- emission helpers ----
            def proj_qk_m(w_sb, b_c, dst, ts, m, chunk=99):
                """One [128,512] psum tile of the Q/K projection; yields every
                `chunk` contraction matmuls."""
                t0 = ts * 512
                ps = mp.tile([128, 512], F32, name="proj_ps", tag="mp")
                for k in range(8):
                    nc.tensor.matmul(
                        ps, lhsT=w_sb[:, k, m * 128:(m + 1) * 128],
                        rhs=xt[:, ts, k, :],
                        start=(k == 0), stop=(k == 7))
                    if k % chunk == chunk - 1 and k < 7:
                        yield
                nc.vector.tensor_scalar_add(
                    dst[:, m, t0:t0 + 512], ps, b_c[:, m:m + 1])
                yield

            def proj_v_tt(g):
                """V projection for t-subtile g (128 rows): 8 matmuls + the
                scatter into vaug."""
                ps = mp.tile([128, 256], F32, name="v_ps", tag="mp")
                for k in range(8):
                    nc.tensor.matmul(
                        ps, lhsT=xt[:, g // 4, k, (g % 4) * 128:(g % 4 + 1) * 128],
                        rhs=wvt[:, k, :], start=(k == 0), stop=(k == 7))
                    if k == 3:
                        yield
                # scatter into vaug: head h -> cols [128h+64, 128h+128)
                nc.vector.tensor_copy(
                    vaug[:, g, :].rearrange("p (h c) -> p h c", h=NH)[:, :, HD:],
                    ps.rearrange("p (h c) -> p h c", h=NH))
                yield

            def gen_proj_qk(ts):
                for w_sb, b_c, dst in ((wqt, bqc, qt), (wkt, bkc, kt)):
                    for m in range(2):
                        yield from proj_qk_m(w_sb, b_c, dst, ts, m, chunk=2)

            def gen_proj_v(ts):
                for tt in range(4):
                    yield from proj_v_tt(4 * ts + tt)

            def gen_wo(j):
                q0 = j * QB
                last = j == NJ - 1
                ob = op.tile([128, 4, DM], BF16, name="ob", tag="ob")
                for qq in range(4):
                    row = q0 + qq * 128
                    for n in range(2):
                        ps = mp.tile([128, 512], F32, name="wo_ps", tag="mp")
                        for kk in range(2):
                            nc.tensor.matmul(
                                ps, lhsT=ot_all[:, kk, row:row + 128],
                                rhs=wot[:, kk, n * 512:(n + 1) * 512],
                                start=(kk == 0), stop=(kk == 1))
                        nc.vector.tensor_copy(ob[:, qq, n * 512:(n + 1) * 512],
                                              ps)
                        yield
                    if last:
                        # tail round: stream the output per 128-row chunk so
                        # the final DMA only trails the last copy
                        dma_eng = nc.sync if qq % 2 == 0 else nc.gpsimd
                        dma_eng.dma_start(
                            out=p_out[row:row + 128, :], in_=ob[:, qq, :])
                if not last:
                    dma_eng = nc.sync if j % 2 == 0 else nc.gpsimd
                    dma_eng.dma_start(
                        out=p_out[q0:q0 + 512, :].rearrange(
                            "(q p) d -> p q d", p=128),
                        in_=ob)
                yield

            # Global filler queue: (key, generator), drained strictly FIFO.
            fillers = deque()

            def fill(n):
                while n > 0 and fillers:
                    try:
                        next(fillers[0][1])
                        n -= 1
                    except StopIteration:
                        fillers.popleft()

            def force(key):
                """Drain fillers until generator `key` has been exhausted."""
                while any(k == key for k, _ in fillers):
                    try:
                        next(fillers[0][1])
                    except StopIteration:
                        fillers.popleft()

            def attention(j, fill_fn):
                q0 = j * QB
                nk = 4 * (j + 1)           # k-tiles of 128
                for H in range(2):          # head pair (2H, 2H+1)
                    ot = [tp.tile([128, QB], F32, name=f"ot{hp}", tag=f"ot{hp}")
                          for hp in range(2)]
                    for ktile in range(nk):
                        s = ktile - 4 * j       # >=0 on diag block
                        c0 = 128 * s if s >= 0 else 0
                        st = sp.tile([128, 2, 512], F32, name="st", tag="st")
                        e_t = ep.tile([128, 2, QB], BF16, name="e", tag="e")
                        for hp in range(2):
                            h = 2 * H + hp
                            r0 = (HD * h) % 128
                            mi = (HD * h) // 128
                            nc.tensor.matmul(
                                st[:, hp, c0:512],
                                lhsT=kt[r0:r0 + HD, mi,
                                        ktile * 128:(ktile + 1) * 128],
                                rhs=qt[r0:r0 + HD, mi, q0 + c0:q0 + QB],
                                start=True, stop=True)
                        nc.scalar.activation(
                            out=e_t, in_=st, func=EXP, scale=1.0)
                        if s >= 0:
                            # mask both heads' diag tile in one strided op
                            dg = e_t[:, :, c0:c0 + 128]
                            trb = bass.AP(
                                tensor=tri.tensor, offset=tri.offset,
                                ap=[tri.ap[0], [0, 2], [1, 128]])
                            nc.vector.tensor_mul(dg, dg, trb)
                        fill_fn(j, H, ktile)
                        for hp in range(2):
                            h = 2 * H + hp
                            nc.tensor.matmul(
                                ot[hp][:, c0:QB],
                                lhsT=vaug[:, ktile, 128 * h:128 * (h + 1)],
                                rhs=e_t[:, hp, c0:QB],
                                start=(ktile == 0), stop=(ktile == nk - 1))
                    for hp in range(2):
                        h = 2 * H + hp
                        rec = bp.tile([64, QB], F32, name="rec", tag="rec")
                        nc.vector.reciprocal_approx_fast(rec, ot[hp][0:64, :])
                        r0 = (HD * h) % 128
                        mi = (HD * h) // 128
                        nc.vector.tensor_mul(
                            ot_all[r0:r0 + HD, mi, q0:q0 + QB],
                            ot[hp][64:128, :], rec)

            # ---- schedule ----
            # Round 0 prefix: Q m0, K m0, V g=0 -> attention(0) H0 can start;
            # the rest of proj(0) rides along as round-0 filler.
            for _ in proj_qk_m(wqt, bqc, qt, 0, 0):
                pass
            for _ in proj_qk_m(wkt, bkc, kt, 0, 0):
                pass
            for _ in proj_v_tt(0):
                pass
            local = deque()
            for g in (proj_v_tt(1), proj_v_tt(2), proj_v_tt(3),
                      proj_qk_m(wqt, bqc, qt, 0, 1, chunk=4),
                      proj_qk_m(wkt, bkc, kt, 0, 1, chunk=4)):
                local.append(("r0", g))

            def fill_r0(j, H, ktile):
                n = 3
                while n > 0 and local:
                    try:
                        next(local[0][1])
                        n -= 1
                    except StopIteration:
                        local.popleft()
                if not local:
                    fill(1)

            for ts in range(1, NJ):
                fillers.append((("qk", ts), gen_proj_qk(ts)))
                fillers.append((("v", ts), gen_proj_v(ts)))

            def fill_main(j, H, ktile):
                if H == 0 and ktile == 4 * j:
                    force(("v", j))   # vaug t-slice j gate (usually a no-op)
                fill(1)

            attention(0, fill_r0)
            while local:
                fill_r0(0, 0, 0)
            for j in range(1, NJ):
                force(("qk", j))
                if j == NJ - 1:
                    # wo is the only filler left whose deadline is the kernel
                    # end; spend it on the most ACT-bound round.
                    for jj in range(NJ - 1):
                        fillers.append((("wo", jj), gen_wo(jj)))
                attention(j, fill_main)
            while fillers:
                fill(64)
            for _ in gen_wo(NJ - 1):
                pass
    nc.compile()
    return nc


def _prep_inputs(x, wq, bq, wk, bk, wv, wo):
    bf = ml_dtypes.bfloat16
    scale = np.float32(1.0 / np.sqrt(HD))
    tri = np.triu(np.ones((128, 128), np.float32)).astype(bf)
    in_maps = []
    for c in range(NCORES):
        b, g = c // 4, c % 4
        sl = slice(DS * g, DS * (g + 1))
        xts = np.ascontiguousarray(
            x[b].T.astype(bf).reshape(8, 128, 4, 512).transpose(2, 1, 0, 3))
        in_maps.append({
            "xt": xts,
            "wqt": np.ascontiguousarray(wq.T[:, sl] * scale).astype(bf),
            "wkt": np.ascontiguousarray(wk.T[:, sl]).astype(bf),
            "wvt": np.ascontiguousarray(wv.T[:, sl]).astype(bf),
            "wot": np.ascontiguousarray(wo[:, sl].T).astype(bf),
            "bqc": np.ascontiguousarray(
                (bq[sl] * scale).reshape(2, 128).T).astype(np.float32),
            "bkc": np.ascontiguousarray(
                bk[sl].reshape(2, 128).T).astype(np.float32),
            "tri": tri,
        })
    return in_maps


TRACE = False
TRACE_DIR = None
LAST_RESULT = None


def kernel(x, wq, bq, wk, bk, wv, bv, wo, bo):
    global LAST_RESULT
    x, wq, bq, wk, bk, wv, bv, wo, bo = [
        np.asarray(a, np.float32)
        for a in (x, wq, bq, wk, bk, wv, bv, wo, bo)]
    if "nc" not in _CACHE:
        _CACHE["nc"] = _build()
    nc = _CACHE["nc"]
    in_maps = _prep_inputs(x, wq, bq, wk, bk, wv, wo)
    res = bass_utils.run_bass_kernel_spmd(
        nc, in_maps, core_ids=list(range(NCORES)), trace=TRACE,
        tmpdir=TRACE_DIR)
    LAST_RESULT = res
    # bv commutes through softmax (rows sum to 1): out += bv @ wo^T + bo.
    const_row = (bv.astype(np.float64) @ wo.T.astype(np.float64) +
                 bo.astype(np.float64)).astype(np.float32)
    out = np.empty((2, T, DM), np.float32)
    for b in range(2):
        acc = res.results[4 * b]["p_out"].astype(np.float32)
        for g in range(1, 4):
            acc = acc + res.results[4 * b + g]["p_out"].astype(np.float32)
        out[b] = acc + const_row
    return out
